# revision 2
# baseline (speedup 1.0000x reference)
"""Multi-head attention (B=2, L=2048, D=2048, H=16, Dh=128) on 8 NeuronCores.

Sharding: tensor-parallel over heads (2 heads/core) for QKV projection +
attention (dispatch A), then sequence-parallel final projection (dispatch B,
512 rows of B*L per core). Host does the small reshuffle between dispatches.

Layout strategy (per core, dispatch A):
  - host feeds x^T (D, B*L) so QKV matmuls contract over d on partitions and
    produce q^T/k^T (Dh on partitions) directly — the layout attention wants.
  - scores computed transposed: S^T[kk, l] (keys on partitions), softmax
    without max-subtraction (logits ~ N(0,1); shift by -3 for fp16 headroom),
    exp evicted to fp16.
  - row sums Z via ones-vector matmuls on the PE (cross-partition reduction),
    replicated across partitions with a K=1 matmul; normalization + V-bias
    folded into the PV eviction (out^T layout makes bv per-partition).
  - matmuls in float32r (full PE speed at N=512, ~tf32 accuracy); probs/V in
    fp16 for the PV/Z stage.
"""

import os
import sys

import numpy as np

for _p in ("/opt/trn_rl_repo",):
    if _p not in sys.path:
        sys.path.insert(0, _p)

import concourse.bacc as bacc
import concourse.mybir as mybir
import concourse.tile as tile
from concourse.bass_utils import run_bass_kernel_spmd

P = 128
B, L, D = 2, 2048, 2048
BL = B * L
H, DH = 16, 128
NCORES = 8
HLOC = H // NCORES            # heads per core = 2
DT = D // P                   # d-tiles = 16
NET = 3 * HLOC                # e-tiles per core in dispatch A = 6 (q0,q1,k0,k1,v0,v1)
NLC = L // 512                # l-chunks of 512 per batch = 4
NKK = L // P                  # key tiles per batch = 16
LCB = BL // NCORES            # rows per core in dispatch B = 512

F32 = mybir.dt.float32
F32R = mybir.dt.float32r
F16 = mybir.dt.float16
ACTF = mybir.ActivationFunctionType
EXP_SHIFT = -3.0

_programs = {}

# Results of the last kernel() call when BASS_MHA_TRACE=1 (for test harness).
last_run_info = {}


def _build_a():
    nc = bacc.Bacc(None, target_bir_lowering=False, debug=False)
    xT = nc.dram_tensor("xT", [D, BL], F32R, kind="ExternalInput")
    wqkvT = nc.dram_tensor("wqkvT", [D, NET * P], F32R, kind="ExternalInput")
    bias_qk = nc.dram_tensor("bias_qk", [4, P], F32, kind="ExternalInput")
    bias_v = nc.dram_tensor("bias_v", [HLOC, P], F32, kind="ExternalInput")
    ones16 = nc.dram_tensor("ones16", [P, 1], F16, kind="ExternalInput")
    ones16r = nc.dram_tensor("ones16r", [1, P], F16, kind="ExternalInput")
    ident16 = nc.dram_tensor("ident16", [P, P], F16, kind="ExternalInput")
    outT = nc.dram_tensor("outT", [HLOC * DH, BL], F32, kind="ExternalOutput")

    with tile.TileContext(nc) as tc:
        with (
            tc.tile_pool(name="const", bufs=1) as const,
            tc.tile_pool(name="xs", bufs=2) as xs,
            tc.tile_pool(name="qk", bufs=2) as qkp,
            tc.tile_pool(name="vt", bufs=2) as vtp,
            tc.tile_pool(name="vn", bufs=1) as vnp,
            tc.tile_pool(name="es", bufs=1) as esp,
            tc.tile_pool(name="ev", bufs=3) as evp,
            tc.tile_pool(name="ps", bufs=8, space="PSUM") as ps,
        ):
            w_sb = const.tile([P, DT, NET * P], F32R)
            nc.sync.dma_start(w_sb[:], wqkvT.rearrange("(t p) e -> p t e", p=P))
            bqk_sb = const.tile([P, 4], F32)
            nc.sync.dma_start(bqk_sb[:], bias_qk.rearrange("t p -> p t"))
            bv_sb = const.tile([P, HLOC], F32)
            nc.sync.dma_start(bv_sb[:], bias_v.rearrange("t p -> p t"))
            ones_l = const.tile([P, 1], F16)
            nc.sync.dma_start(ones_l[:], ones16[:])
            ones_r = const.tile([1, P], F16)
            nc.sync.dma_start(ones_r[:], ones16r[:])
            ident = const.tile([P, P], F16)
            nc.sync.dma_start(ident[:], ident16[:])
            shift = const.tile([P, 1], F32)
            nc.any.memset(shift[:], EXP_SHIFT)

            for b in range(B):
                # ---- Phase 1: QKV projection (transposed outputs) ----
                qk_sb = qkp.tile([P, 4, L], F32R, tag="qk")
                vT_sb = vtp.tile([P, HLOC, L], F16, tag="vt")
                for lc in range(NLC):
                    pss = [
                        ps.tile([P, 512], F32, tag="ps", name=f"ps_qkv{et}")
                        for et in range(NET)
                    ]
                    for dh_half in range(2):
                        xt = xs.tile([P, DT // 2, 512], F32R, tag="xs")
                        nc.sync.dma_start(
                            xt[:],
                            xT[
                                dh_half * (D // 2) : (dh_half + 1) * (D // 2),
                                b * L + lc * 512 : b * L + (lc + 1) * 512,
                            ].rearrange("(t p) l -> p t l", p=P),
                        )
                        for d8 in range(DT // 2):
                            d = dh_half * (DT // 2) + d8
                            for et in range(NET):
                                nc.tensor.matmul(
                                    pss[et][:],
                                    w_sb[:, d, et * P : (et + 1) * P],
                                    xt[:, d8, :],
                                    start=(d == 0),
                                    stop=(d == DT - 1),
                                )
                    for et in range(NET):
                        if et < 4:
                            nc.scalar.activation(
                                qk_sb[:, et, lc * 512 : (lc + 1) * 512],
                                pss[et][:],
                                ACTF.Identity,
                                bias=bqk_sb[:, et : et + 1],
                            )
                        else:
                            nc.scalar.activation(
                                vT_sb[:, et - 4, lc * 512 : (lc + 1) * 512],
                                pss[et][:],
                                ACTF.Copy,
                            )

                # ---- Phase 2: attention, per local head ----
                for h in range(HLOC):
                    # transpose v^T (Dh, L) -> v natural tiles (kk, Dh)
                    v_sb = vnp.tile([P, NKK, DH], F16, tag="vn")
                    for kk in range(NKK):
                        pst = ps.tile([P, P], F16, tag="ps")
                        nc.tensor.transpose(
                            pst[:], vT_sb[:, h, kk * P : (kk + 1) * P], ident[:]
                        )
                        nc.vector.tensor_copy(v_sb[:, kk, :], pst[:])

                    for lc in range(NLC):
                        lsl = slice(lc * 512, (lc + 1) * 512)
                        es_sb = esp.tile([P, NKK, 512], F16, tag="es")
                        for kk in range(NKK):
                            ps_s = ps.tile([P, 512], F32, tag="ps")
                            nc.tensor.matmul(
                                ps_s[:],
                                qk_sb[:, 2 + h, kk * P : (kk + 1) * P],
                                qk_sb[:, h, lsl],
                                start=True,
                                stop=True,
                            )
                            nc.scalar.activation(
                                es_sb[:, kk, :], ps_s[:], ACTF.Exp, bias=shift[:]
                            )
                        # Z row-sums (over keys = partitions) via ones matmul
                        ps_z = ps.tile([1, 512], F32, tag="ps")
                        for kk in range(NKK):
                            nc.tensor.matmul(
                                ps_z[:],
                                ones_l[:],
                                es_sb[:, kk, :],
                                start=(kk == 0),
                                stop=(kk == NKK - 1),
                            )
                        z16 = evp.tile([1, 512], F16, tag="z16")
                        nc.vector.tensor_copy(z16[:], ps_z[:])
                        # PV accumulate
                        ps_pv = ps.tile([P, 512], F32, tag="ps")
                        for kk in range(NKK):
                            nc.tensor.matmul(
                                ps_pv[:],
                                v_sb[:, kk, :],
                                es_sb[:, kk, :],
                                start=(kk == 0),
                                stop=(kk == NKK - 1),
                            )
                        # replicate Z across partitions, reciprocal, normalize
                        ps_zb = ps.tile([P, 512], F32, tag="ps")
                        nc.tensor.matmul(
                            ps_zb[:], ones_r[:], z16[:], start=True, stop=True
                        )
                        recip = evp.tile([P, 512], F32, tag="recip")
                        nc.vector.reciprocal(recip[:], ps_zb[:])
                        out_sb = evp.tile([P, 512], F32, tag="out")
                        nc.vector.tensor_tensor(
                            out_sb[:], ps_pv[:], recip[:], mybir.AluOpType.mult
                        )
                        nc.vector.tensor_scalar_add(
                            out_sb[:], out_sb[:], bv_sb[:, h : h + 1]
                        )
                        nc.sync.dma_start(
                            outT[h * DH : (h + 1) * DH, b * L + lc * 512 : b * L + (lc + 1) * 512],
                            out_sb[:],
                        )
    nc.compile()
    return nc


def _build_b():
    nc = bacc.Bacc(None, target_bir_lowering=False, debug=False)
    outTc = nc.dram_tensor("outTc", [D, LCB], F32R, kind="ExternalInput")
    projWT = nc.dram_tensor("projWT", [D, D], F32R, kind="ExternalInput")
    bias_p = nc.dram_tensor("bias_p", [DT, P], F32, kind="ExternalInput")
    finalT = nc.dram_tensor("finalT", [D, LCB], F32, kind="ExternalOutput")

    with tile.TileContext(nc) as tc:
        with (
            tc.tile_pool(name="const", bufs=1) as const,
            tc.tile_pool(name="wp", bufs=3) as wp,
            tc.tile_pool(name="fo", bufs=3) as fo,
            tc.tile_pool(name="ps", bufs=6, space="PSUM") as ps,
        ):
            bias_sb = const.tile([P, DT], F32)
            nc.sync.dma_start(bias_sb[:], bias_p.rearrange("t p -> p t"))
            oc_sb = const.tile([P, DT, LCB], F32R)
            nc.sync.dma_start(oc_sb[:], outTc.rearrange("(t p) l -> p t l", p=P))
            for et in range(DT):
                pw = wp.tile([P, DT, P], F32R, tag="pw")
                nc.sync.dma_start(
                    pw[:],
                    projWT[:, et * P : (et + 1) * P].rearrange(
                        "(t p) e -> p t e", p=P
                    ),
                )
                acc = ps.tile([P, LCB], F32, tag="ps")
                for d in range(DT):
                    nc.tensor.matmul(
                        acc[:],
                        pw[:, d, :],
                        oc_sb[:, d, :],
                        start=(d == 0),
                        stop=(d == DT - 1),
                    )
                f_sb = fo.tile([P, LCB], F32, tag="f")
                nc.scalar.activation(
                    f_sb[:], acc[:], ACTF.Identity, bias=bias_sb[:, et : et + 1]
                )
                nc.sync.dma_start(finalT[et * P : (et + 1) * P, :], f_sb[:])
    nc.compile()
    return nc


def _get_programs():
    if "a" not in _programs:
        _programs["a"] = _build_a()
        _programs["b"] = _build_b()
    return _programs["a"], _programs["b"]


def kernel(x, Wqkv_w, Wqkv_b, proj_w, proj_b):
    x = np.ascontiguousarray(np.asarray(x, dtype=np.float32))
    Wqkv_w = np.asarray(Wqkv_w, dtype=np.float32)
    Wqkv_b = np.asarray(Wqkv_b, dtype=np.float32)
    proj_w = np.asarray(proj_w, dtype=np.float32)
    proj_b = np.asarray(proj_b, dtype=np.float32)

    nc_a, nc_b = _get_programs()
    trace = bool(int(os.environ.get("BASS_MHA_TRACE", "0")))
    qscale = np.float32(1.0 / np.sqrt(DH))

    xT = np.ascontiguousarray(x.reshape(BL, D).T)
    ones16 = np.ones((P, 1), np.float16)
    ones16r = np.ones((1, P), np.float16)
    ident16 = np.eye(P, dtype=np.float16)

    in_maps_a = []
    for c in range(NCORES):
        g0 = HLOC * c
        rows = []
        biases_qk = np.empty((4, P), np.float32)
        for j in range(HLOC):
            rows.append(Wqkv_w[(g0 + j) * DH : (g0 + j + 1) * DH] * qscale)
            biases_qk[j] = Wqkv_b[(g0 + j) * DH : (g0 + j + 1) * DH] * qscale
        for j in range(HLOC):
            rows.append(Wqkv_w[D + (g0 + j) * DH : D + (g0 + j + 1) * DH])
            biases_qk[HLOC + j] = Wqkv_b[D + (g0 + j) * DH : D + (g0 + j + 1) * DH]
        bias_v = np.empty((HLOC, P), np.float32)
        for j in range(HLOC):
            rows.append(Wqkv_w[2 * D + (g0 + j) * DH : 2 * D + (g0 + j + 1) * DH])
            bias_v[j] = Wqkv_b[2 * D + (g0 + j) * DH : 2 * D + (g0 + j + 1) * DH]
        wqkvT = np.ascontiguousarray(np.concatenate(rows, axis=0).T)
        in_maps_a.append(
            {
                "xT": xT,
                "wqkvT": wqkvT,
                "bias_qk": biases_qk,
                "bias_v": bias_v,
                "ones16": ones16,
                "ones16r": ones16r,
                "ident16": ident16,
            }
        )

    res_a = run_bass_kernel_spmd(nc_a, in_maps_a, list(range(NCORES)), trace=trace)
    outT_full = np.concatenate(
        [res_a.results[c]["outT"] for c in range(NCORES)], axis=0
    )  # (D, BL)

    projWT = np.ascontiguousarray(proj_w.T)
    bias_p = np.ascontiguousarray(proj_b.reshape(DT, P))
    in_maps_b = [
        {
            "outTc": np.ascontiguousarray(outT_full[:, c * LCB : (c + 1) * LCB]),
            "projWT": projWT,
            "bias_p": bias_p,
        }
        for c in range(NCORES)
    ]
    res_b = run_bass_kernel_spmd(nc_b, in_maps_b, list(range(NCORES)), trace=trace)
    finalT = np.concatenate(
        [res_b.results[c]["finalT"] for c in range(NCORES)], axis=1
    )  # (D, BL)

    if trace:
        last_run_info["a"] = res_a
        last_run_info["b"] = res_b

    return np.ascontiguousarray(finalT.T).reshape(B, L, D)


# revision 3
# speedup vs baseline: 1.0830x; 1.0830x over previous
"""Multi-head attention (B=2, L=2048, D=2048, H=16, Dh=128) on 8 NeuronCores.

Sharding: tensor-parallel over heads (2 heads/core) for QKV projection +
attention (dispatch A), then sequence-parallel final projection (dispatch B,
512 rows of B*L per core). Host does the small reshuffle between dispatches.

Layout strategy (per core, dispatch A):
  - host feeds x^T (D, B*L) so QKV matmuls contract over d on partitions and
    produce q^T/k^T (Dh on partitions) directly — the layout attention wants.
  - scores computed transposed: S^T[kk, l] (keys on partitions), softmax
    without max-subtraction (logits ~ N(0,1); shift by -3 for fp16 headroom),
    exp evicted to fp16.
  - row sums Z via ones-vector matmuls on the PE (cross-partition reduction),
    replicated across partitions with a K=1 matmul; normalization + V-bias
    folded into the PV eviction (out^T layout makes bv per-partition).
  - matmuls in float32r (full PE speed at N=512, ~tf32 accuracy); probs/V in
    fp16 for the PV/Z stage.
"""

import os
import sys

import numpy as np

for _p in ("/opt/trn_rl_repo",):
    if _p not in sys.path:
        sys.path.insert(0, _p)

import concourse.bacc as bacc
import concourse.mybir as mybir
import concourse.tile as tile
from concourse.bass_utils import run_bass_kernel_spmd

P = 128
B, L, D = 2, 2048, 2048
BL = B * L
H, DH = 16, 128
NCORES = 8
HLOC = H // NCORES            # heads per core = 2
DT = D // P                   # d-tiles = 16
NET = 3 * HLOC                # e-tiles per core in dispatch A = 6 (q0,q1,k0,k1,v0,v1)
NLC = L // 512                # l-chunks of 512 per batch = 4
NKK = L // P                  # key tiles per batch = 16
LCB = BL // NCORES            # rows per core in dispatch B = 512

F32 = mybir.dt.float32
F32R = mybir.dt.float32r
F16 = mybir.dt.float16
# matmul storage dtype: fp16 (full PE speed, 10-bit mantissa) unless overridden
MM_DT = F32R if os.environ.get("BASS_MHA_F32R") else F16
MM_NP = np.float32 if os.environ.get("BASS_MHA_F32R") else np.float16
ACTF = mybir.ActivationFunctionType
EXP_SHIFT = -3.0

_programs = {}

# Results of the last kernel() call when BASS_MHA_TRACE=1 (for test harness).
last_run_info = {}


def _build_a():
    nc = bacc.Bacc(None, target_bir_lowering=False, debug=False)
    xT = nc.dram_tensor("xT", [D, BL], MM_DT, kind="ExternalInput")
    wqkvT = nc.dram_tensor("wqkvT", [D, NET * P], MM_DT, kind="ExternalInput")
    bias_qk = nc.dram_tensor("bias_qk", [4, P], F32, kind="ExternalInput")
    bias_v = nc.dram_tensor("bias_v", [HLOC, P], F32, kind="ExternalInput")
    ones16 = nc.dram_tensor("ones16", [P, 1], F16, kind="ExternalInput")
    ones16r = nc.dram_tensor("ones16r", [1, P], F16, kind="ExternalInput")
    ident16 = nc.dram_tensor("ident16", [P, P], F16, kind="ExternalInput")
    outT = nc.dram_tensor("outT", [HLOC * DH, BL], F32, kind="ExternalOutput")

    with tile.TileContext(nc) as tc:
        with (
            tc.tile_pool(name="const", bufs=1) as const,
            tc.tile_pool(name="xs", bufs=2) as xs,
            tc.tile_pool(name="qk", bufs=2) as qkp,
            tc.tile_pool(name="vt", bufs=2) as vtp,
            tc.tile_pool(name="vn", bufs=1) as vnp,
            tc.tile_pool(name="es", bufs=1) as esp,
            tc.tile_pool(name="ev", bufs=3) as evp,
            tc.tile_pool(name="ps", bufs=8, space="PSUM") as ps,
        ):
            w_sb = const.tile([P, DT, NET * P], MM_DT)
            nc.sync.dma_start(w_sb[:], wqkvT.rearrange("(t p) e -> p t e", p=P))
            bqk_sb = const.tile([P, 4], F32)
            nc.sync.dma_start(bqk_sb[:], bias_qk.rearrange("t p -> p t"))
            bv_sb = const.tile([P, HLOC], F32)
            nc.sync.dma_start(bv_sb[:], bias_v.rearrange("t p -> p t"))
            ones_l = const.tile([P, 1], F16)
            nc.sync.dma_start(ones_l[:], ones16[:])
            ones_r = const.tile([1, P], F16)
            nc.sync.dma_start(ones_r[:], ones16r[:])
            ident = const.tile([P, P], F16)
            nc.sync.dma_start(ident[:], ident16[:])
            shift = const.tile([P, 1], F32)
            nc.any.memset(shift[:], EXP_SHIFT)

            for b in range(B):
                # ---- Phase 1: QKV projection (transposed outputs) ----
                qk_sb = qkp.tile([P, 4, L], MM_DT, tag="qk")
                vT_sb = vtp.tile([P, HLOC, L], F16, tag="vt")
                for lc in range(NLC):
                    pss = [
                        ps.tile([P, 512], F32, tag="ps", name=f"ps_qkv{et}")
                        for et in range(NET)
                    ]
                    for dh_half in range(2):
                        xt = xs.tile([P, DT // 2, 512], MM_DT, tag="xs")
                        nc.sync.dma_start(
                            xt[:],
                            xT[
                                dh_half * (D // 2) : (dh_half + 1) * (D // 2),
                                b * L + lc * 512 : b * L + (lc + 1) * 512,
                            ].rearrange("(t p) l -> p t l", p=P),
                        )
                        for d8 in range(DT // 2):
                            d = dh_half * (DT // 2) + d8
                            for et in range(NET):
                                nc.tensor.matmul(
                                    pss[et][:],
                                    w_sb[:, d, et * P : (et + 1) * P],
                                    xt[:, d8, :],
                                    start=(d == 0),
                                    stop=(d == DT - 1),
                                )
                    for et in range(NET):
                        if et < 4:
                            nc.scalar.activation(
                                qk_sb[:, et, lc * 512 : (lc + 1) * 512],
                                pss[et][:],
                                ACTF.Identity,
                                bias=bqk_sb[:, et : et + 1],
                            )
                        else:
                            nc.scalar.activation(
                                vT_sb[:, et - 4, lc * 512 : (lc + 1) * 512],
                                pss[et][:],
                                ACTF.Copy,
                            )

                # ---- Phase 2: attention, per local head ----
                for h in range(HLOC):
                    # transpose v^T (Dh, L) -> v natural tiles (kk, Dh)
                    v_sb = vnp.tile([P, NKK, DH], F16, tag="vn")
                    for kk in range(NKK):
                        pst = ps.tile([P, P], F16, tag="ps")
                        nc.tensor.transpose(
                            pst[:], vT_sb[:, h, kk * P : (kk + 1) * P], ident[:]
                        )
                        nc.vector.tensor_copy(v_sb[:, kk, :], pst[:])

                    for lc in range(NLC):
                        lsl = slice(lc * 512, (lc + 1) * 512)
                        es_sb = esp.tile([P, NKK, 512], F16, tag="es")
                        for kk in range(NKK):
                            ps_s = ps.tile([P, 512], F32, tag="ps")
                            nc.tensor.matmul(
                                ps_s[:],
                                qk_sb[:, 2 + h, kk * P : (kk + 1) * P],
                                qk_sb[:, h, lsl],
                                start=True,
                                stop=True,
                            )
                            nc.scalar.activation(
                                es_sb[:, kk, :], ps_s[:], ACTF.Exp, bias=shift[:]
                            )
                        # Z row-sums (over keys = partitions) via ones matmul
                        ps_z = ps.tile([1, 512], F32, tag="ps")
                        for kk in range(NKK):
                            nc.tensor.matmul(
                                ps_z[:],
                                ones_l[:],
                                es_sb[:, kk, :],
                                start=(kk == 0),
                                stop=(kk == NKK - 1),
                            )
                        z16 = evp.tile([1, 512], F16, tag="z16")
                        nc.vector.tensor_copy(z16[:], ps_z[:])
                        # PV accumulate
                        ps_pv = ps.tile([P, 512], F32, tag="ps")
                        for kk in range(NKK):
                            nc.tensor.matmul(
                                ps_pv[:],
                                v_sb[:, kk, :],
                                es_sb[:, kk, :],
                                start=(kk == 0),
                                stop=(kk == NKK - 1),
                            )
                        # replicate Z across partitions, reciprocal, normalize
                        ps_zb = ps.tile([P, 512], F32, tag="ps")
                        nc.tensor.matmul(
                            ps_zb[:], ones_r[:], z16[:], start=True, stop=True
                        )
                        recip = evp.tile([P, 512], F32, tag="recip")
                        nc.vector.reciprocal(recip[:], ps_zb[:])
                        out_sb = evp.tile([P, 512], F32, tag="out")
                        nc.vector.tensor_tensor(
                            out_sb[:], ps_pv[:], recip[:], mybir.AluOpType.mult
                        )
                        nc.vector.tensor_scalar_add(
                            out_sb[:], out_sb[:], bv_sb[:, h : h + 1]
                        )
                        nc.sync.dma_start(
                            outT[h * DH : (h + 1) * DH, b * L + lc * 512 : b * L + (lc + 1) * 512],
                            out_sb[:],
                        )
    nc.compile()
    return nc


def _build_b():
    nc = bacc.Bacc(None, target_bir_lowering=False, debug=False)
    outTc = nc.dram_tensor("outTc", [D, LCB], MM_DT, kind="ExternalInput")
    projWT = nc.dram_tensor("projWT", [D, D], MM_DT, kind="ExternalInput")
    bias_p = nc.dram_tensor("bias_p", [DT, P], F32, kind="ExternalInput")
    finalT = nc.dram_tensor("finalT", [D, LCB], F32, kind="ExternalOutput")

    with tile.TileContext(nc) as tc:
        with (
            tc.tile_pool(name="const", bufs=1) as const,
            tc.tile_pool(name="wp", bufs=3) as wp,
            tc.tile_pool(name="fo", bufs=3) as fo,
            tc.tile_pool(name="ps", bufs=6, space="PSUM") as ps,
        ):
            bias_sb = const.tile([P, DT], F32)
            nc.sync.dma_start(bias_sb[:], bias_p.rearrange("t p -> p t"))
            oc_sb = const.tile([P, DT, LCB], MM_DT)
            nc.sync.dma_start(oc_sb[:], outTc.rearrange("(t p) l -> p t l", p=P))
            for et in range(DT):
                pw = wp.tile([P, DT, P], MM_DT, tag="pw")
                nc.sync.dma_start(
                    pw[:],
                    projWT[:, et * P : (et + 1) * P].rearrange(
                        "(t p) e -> p t e", p=P
                    ),
                )
                acc = ps.tile([P, LCB], F32, tag="ps")
                for d in range(DT):
                    nc.tensor.matmul(
                        acc[:],
                        pw[:, d, :],
                        oc_sb[:, d, :],
                        start=(d == 0),
                        stop=(d == DT - 1),
                    )
                f_sb = fo.tile([P, LCB], F32, tag="f")
                nc.scalar.activation(
                    f_sb[:], acc[:], ACTF.Identity, bias=bias_sb[:, et : et + 1]
                )
                nc.sync.dma_start(finalT[et * P : (et + 1) * P, :], f_sb[:])
    nc.compile()
    return nc


def _get_programs():
    if "a" not in _programs:
        _programs["a"] = _build_a()
        _programs["b"] = _build_b()
    return _programs["a"], _programs["b"]


def kernel(x, Wqkv_w, Wqkv_b, proj_w, proj_b):
    x = np.ascontiguousarray(np.asarray(x, dtype=np.float32))
    Wqkv_w = np.asarray(Wqkv_w, dtype=np.float32)
    Wqkv_b = np.asarray(Wqkv_b, dtype=np.float32)
    proj_w = np.asarray(proj_w, dtype=np.float32)
    proj_b = np.asarray(proj_b, dtype=np.float32)

    nc_a, nc_b = _get_programs()
    trace = bool(int(os.environ.get("BASS_MHA_TRACE", "0")))
    qscale = np.float32(1.0 / np.sqrt(DH))

    xT = np.ascontiguousarray(x.reshape(BL, D).T).astype(MM_NP)
    ones16 = np.ones((P, 1), np.float16)
    ones16r = np.ones((1, P), np.float16)
    ident16 = np.eye(P, dtype=np.float16)

    in_maps_a = []
    for c in range(NCORES):
        g0 = HLOC * c
        rows = []
        biases_qk = np.empty((4, P), np.float32)
        for j in range(HLOC):
            rows.append(Wqkv_w[(g0 + j) * DH : (g0 + j + 1) * DH] * qscale)
            biases_qk[j] = Wqkv_b[(g0 + j) * DH : (g0 + j + 1) * DH] * qscale
        for j in range(HLOC):
            rows.append(Wqkv_w[D + (g0 + j) * DH : D + (g0 + j + 1) * DH])
            biases_qk[HLOC + j] = Wqkv_b[D + (g0 + j) * DH : D + (g0 + j + 1) * DH]
        bias_v = np.empty((HLOC, P), np.float32)
        for j in range(HLOC):
            rows.append(Wqkv_w[2 * D + (g0 + j) * DH : 2 * D + (g0 + j + 1) * DH])
            bias_v[j] = Wqkv_b[2 * D + (g0 + j) * DH : 2 * D + (g0 + j + 1) * DH]
        wqkvT = np.ascontiguousarray(np.concatenate(rows, axis=0).T).astype(MM_NP)
        in_maps_a.append(
            {
                "xT": xT,
                "wqkvT": wqkvT,
                "bias_qk": biases_qk,
                "bias_v": bias_v,
                "ones16": ones16,
                "ones16r": ones16r,
                "ident16": ident16,
            }
        )

    res_a = run_bass_kernel_spmd(nc_a, in_maps_a, list(range(NCORES)), trace=trace)
    outT_full = np.concatenate(
        [res_a.results[c]["outT"] for c in range(NCORES)], axis=0
    )  # (D, BL)

    projWT = np.ascontiguousarray(proj_w.T).astype(MM_NP)
    bias_p = np.ascontiguousarray(proj_b.reshape(DT, P))
    in_maps_b = [
        {
            "outTc": np.ascontiguousarray(outT_full[:, c * LCB : (c + 1) * LCB]).astype(MM_NP),
            "projWT": projWT,
            "bias_p": bias_p,
        }
        for c in range(NCORES)
    ]
    res_b = run_bass_kernel_spmd(nc_b, in_maps_b, list(range(NCORES)), trace=trace)
    finalT = np.concatenate(
        [res_b.results[c]["finalT"] for c in range(NCORES)], axis=1
    )  # (D, BL)

    if trace:
        last_run_info["a"] = res_a
        last_run_info["b"] = res_b

    return np.ascontiguousarray(finalT.T).reshape(B, L, D)
